# revision 1
# baseline (speedup 1.0000x reference)
"""Distributed Trainium2 Bass kernel for the GAT-Actor (gnn_message_passing).

Strategy (8 NeuronCores, 1-D node partition):
  - nodes sharded contiguously: core i owns rows [i*NLOC, (i+1)*NLOC)
  - edges assigned to the core owning their DESTINATION node
  - per-core: h = x_shard @ W; row table Ht[n] = [h_n (128 f32) | e_src_n | pad]
    AllGather -> full table in HBM (38 MB); e_dst table stays local
  - edge stage: edges sorted by dst into 128-node chunks; dma_gather pulls
    768B rows by src; e_dst[dst_e] is reconstructed on-device per edge via
    onehot(iota==dst_rel) x broadcast(e_dst row) with a fused
    scalar_tensor_tensor accum (no per-edge e_dst gather);
    w = exp(leaky(e_src+e_dst)); a fused DVE tensor_scalar builds
    S[e, d] = (iota[d] == dst_rel_e) * w_e and TensorE accumulates
    out^T[h, d] += rows^T S and denom[d] += 1^T S in PSUM (exact f32).
    (dma_scatter_add is NOT used: its CCE read-modify-write races on duplicate
    indices - verified on hardware.)
  - tail: normalize by denom, +b_gat, relu; BN stats via 1KB AllReduce folded
    into rescaled fc weights; fc1/fc2/fc3 on TensorE; row softmax; per-core
    [NLOC, 32] output shards concatenated on host.
"""

import os
import sys

for _p in ("/opt/trn_rl_repo", "/root/.axon_site/_ro/trn_rl_repo"):
    if os.path.isdir(_p) and _p not in sys.path:
        sys.path.insert(0, _p)

import numpy as np

from concourse import bass, bacc, tile, mybir
from concourse.bass_utils import run_bass_kernel_spmd

f32 = mybir.dt.float32
i16 = mybir.dt.int16
AF = mybir.ActivationFunctionType
ALU = mybir.AluOpType
f32r = mybir.dt.float32r

NCORES = 8
CHUNK_C = 128          # node-chunk width for PSUM aggregation
NEG_SLOPE = 0.2
EPS = 1e-5

_cache = {}
last_results = None    # BassKernelResults of the most recent run (for profiling)


# --------------------------------------------------------------------------
# host-side edge preprocessing
# --------------------------------------------------------------------------

def _wrap_idx(idx):
    """int16 index stream -> [128, len/16]: idx j at partition j%16, col j//16,
    replicated across the 8 gpsimd core groups."""
    idx = np.asarray(idx, np.int16)
    m = idx.shape[0]
    assert m % 16 == 0
    arr = idx.reshape(m // 16, 16).T
    return np.ascontiguousarray(np.tile(arr, (8, 1)))


def _prep_edges(edge_index, N, NLOC, TH, C):
    """Group edges per core by (dst-chunk, src-half); pad each group to a
    block count shared across cores. Returns per-core index streams and the
    compile-time block counts."""
    src = np.asarray(edge_index[0], np.int64)
    dst = np.asarray(edge_index[1], np.int64)
    NCH = -(-NLOC // C)

    cores = []
    counts = np.zeros((NCORES, NCH, 2), np.int64)
    for i in range(NCORES):
        sel = (dst // NLOC) == i
        s = src[sel]
        d = dst[sel] - i * NLOC
        ch = d // C
        hf = (s >= TH).astype(np.int64)
        order = np.lexsort((hf, ch))
        s, d, ch, hf = s[order], d[order], ch[order], hf[order]
        for c in range(NCH):
            for h in range(2):
                counts[i, c, h] = np.count_nonzero((ch == c) & (hf == h))
        cores.append((s, d, ch, hf))

    NA = [int(-(-counts[:, c, 0].max() // 128)) for c in range(NCH)]
    NB = [int(-(-counts[:, c, 1].max() // 128)) for c in range(NCH)]
    for c in range(NCH):
        if NA[c] + NB[c] == 0:
            NA[c] = 1

    TOTB = sum(NA) + sum(NB)
    TOTE = TOTB * 128

    per_core = []
    for i in range(NCORES):
        s, d, ch, hf = cores[i]
        src_idx = np.zeros(TOTE, np.int16)
        dst_rel = np.full(TOTE, -1.0, np.float32)
        pos = 0
        ptr = 0
        for c in range(NCH):
            for h, nblk in ((0, NA[c]), (1, NB[c])):
                cnt = int(counts[i, c, h])
                sl = slice(ptr, ptr + cnt)
                ss, dd = s[sl], d[sl]
                ptr += cnt
                if cnt:
                    src_idx[pos:pos + cnt] = (ss - (TH if h else 0)).astype(np.int16)
                    dst_rel[pos:pos + cnt] = (dd - c * C).astype(np.float32)
                pos += nblk * 128
        assert ptr == len(s)
        # dst_rel layout: edge j -> partition j%128, col j//128
        dst_rel_t = np.ascontiguousarray(dst_rel.reshape(TOTB, 128).T)
        per_core.append({
            "src_idx": _wrap_idx(src_idx),
            "dst_rel": dst_rel_t,
        })
    return per_core, NA, NB


# --------------------------------------------------------------------------
# device graph
# --------------------------------------------------------------------------

def _build_nc(N, D, H, A, NLOC, TH, C, NA, NB):
    KD = D // 128
    NT = -(-NLOC // 128)          # 128-node tiles
    NLOCP = NT * 128
    NCH = len(NA)
    TOTB = sum(NA) + sum(NB)
    NBFmax = max(NA[c] + NB[c] for c in range(NCH))
    ROWW = 192                    # row width of the gathered table (f32)
    split = TH < N                # whether a B-half table exists

    nc = bacc.Bacc("TRN2", num_devices=NCORES)

    # ---- inputs
    x_sh = nc.dram_tensor("x_shard", [NLOC, D], f32, kind="ExternalInput")
    W_in = nc.dram_tensor("W", [D, H], f32, kind="ExternalInput")
    asrcb = nc.dram_tensor("asrc_b", [128, H], f32, kind="ExternalInput")
    adstb = nc.dram_tensor("adst_b", [128, H], f32, kind="ExternalInput")
    bgat = nc.dram_tensor("b_gat", [H, 1], f32, kind="ExternalInput")
    bn0p = nc.dram_tensor("bn0p", [H, 2], f32, kind="ExternalInput")   # g0, beta0
    bn2p = nc.dram_tensor("bn2p", [H, 2], f32, kind="ExternalInput")   # g2, beta2
    W1_in = nc.dram_tensor("W1", [H, H], f32, kind="ExternalInput")
    b1_in = nc.dram_tensor("b1", [H, 1], f32, kind="ExternalInput")
    W2_in = nc.dram_tensor("W2", [H, H], f32, kind="ExternalInput")
    b2_in = nc.dram_tensor("b2", [H, 1], f32, kind="ExternalInput")
    W3_in = nc.dram_tensor("W3", [H, A], f32, kind="ExternalInput")
    b3_in = nc.dram_tensor("b3", [A, 1], f32, kind="ExternalInput")
    ident_in = nc.dram_tensor("ident", [128, 128], f32, kind="ExternalInput")
    iota_in = nc.dram_tensor("iota_b", [128, C], f32, kind="ExternalInput")
    onescol_in = nc.dram_tensor("ones_col", [128, 1], f32, kind="ExternalInput")
    onesrow_in = nc.dram_tensor("ones_row", [1, 128], f32, kind="ExternalInput")
    srci_in = nc.dram_tensor("src_idx", [128, TOTB * 8], i16, kind="ExternalInput")
    dstr_in = nc.dram_tensor("dst_rel", [128, TOTB], f32, kind="ExternalInput")

    out_t = nc.dram_tensor("out", [NLOC, A], f32, kind="ExternalOutput")

    with tile.TileContext(nc) as tc:
        with tc.tile_pool(name="const", bufs=1) as cp, \
             tc.tile_pool(name="dram", bufs=1, space="DRAM") as dram, \
             tc.tile_pool(name="big", bufs=1) as bigp:

            # ---- load constants
            W_sb = cp.tile([128, KD, H], f32)
            nc.sync.dma_start(W_sb[:], bass.AP(W_in, 0, [[H, 128], [128 * H, KD], [1, H]]))
            ident = cp.tile([128, 128], f32)
            nc.sync.dma_start(ident[:], ident_in[:])
            asrc_sb = cp.tile([128, H], f32)
            nc.sync.dma_start(asrc_sb[:], asrcb[:])
            adst_sb = cp.tile([128, H], f32)
            nc.sync.dma_start(adst_sb[:], adstb[:])
            bgat_sb = cp.tile([H, 1], f32)
            nc.sync.dma_start(bgat_sb[:], bgat[:])
            bn0_sb = cp.tile([H, 2], f32)
            nc.sync.dma_start(bn0_sb[:], bn0p[:])
            bn2_sb = cp.tile([H, 2], f32)
            nc.sync.dma_start(bn2_sb[:], bn2p[:])
            W1_sb = cp.tile([H, H], f32)
            nc.sync.dma_start(W1_sb[:], W1_in[:])
            b1_sb = cp.tile([H, 1], f32)
            nc.sync.dma_start(b1_sb[:], b1_in[:])
            W2_sb = cp.tile([H, H], f32)
            nc.sync.dma_start(W2_sb[:], W2_in[:])
            b2_sb = cp.tile([H, 1], f32)
            nc.sync.dma_start(b2_sb[:], b2_in[:])
            W3_sb = cp.tile([H, A], f32)
            nc.sync.dma_start(W3_sb[:], W3_in[:])
            b3_sb = cp.tile([A, 1], f32)
            nc.sync.dma_start(b3_sb[:], b3_in[:])
            iota_sb = cp.tile([128, C], f32)
            nc.sync.dma_start(iota_sb[:], iota_in[:])
            onesc = cp.tile([128, 1], f32)
            nc.sync.dma_start(onesc[:], onescol_in[:])
            onesr = cp.tile([1, 128], f32)
            nc.sync.dma_start(onesr[:], onesrow_in[:])
            srci_sb = bigp.tile([128, TOTB * 8], i16)
            nc.sync.dma_start(srci_sb[:], srci_in[:])
            dstr_sb = bigp.tile([128, TOTB], f32)
            nc.sync.dma_start(dstr_sb[:], dstr_in[:])

            # ---- internal DRAM
            hloc_d = dram.tile([NLOC, ROWW], f32)
            hfull_d = dram.tile([N, ROWW], f32, addr_space="Shared")
            bn_in_0 = dram.tile([H, 2], f32)
            bn_out_0 = dram.tile([H, 2], f32, addr_space="Shared")
            bn_in_1 = dram.tile([H, 2], f32)
            bn_out_1 = dram.tile([H, 2], f32, addr_space="Shared")

            # ================= stage 1: h, e_src, e_dst ====================
            s1big_cm = tc.tile_pool(name="s1big", bufs=1)
            s1big = s1big_cm.__enter__()
            h_sb = s1big.tile([128, NT, ROWW], f32)
            nc.vector.memset(h_sb[:], 0.0)
            edstloc = bigp.tile([128, NT], f32)
            with tc.tile_pool(name="s1", bufs=3) as s1p, \
                 tc.tile_pool(name="s1ps", bufs=2, space="PSUM") as s1ps:
                for t in range(NT):
                    rows = min(128, NLOC - t * 128)
                    x_t = s1p.tile([128, D], f32, tag="xt")
                    nc.sync.dma_start(x_t[:rows, :], x_sh[t * 128: t * 128 + rows, :])
                    h_ps = s1ps.tile([128, H], f32, tag="hps")
                    for k in range(KD):
                        xT_ps = s1ps.tile([128, 128], f32, tag="xTps")
                        nc.tensor.transpose(xT_ps[:], x_t[:, k * 128:(k + 1) * 128], ident[:])
                        xT_sb = s1p.tile([128, 128], f32, tag="xTsb")
                        nc.vector.tensor_copy(xT_sb[:], xT_ps[:])
                        nc.tensor.matmul(h_ps[:], xT_sb[:], W_sb[:, k, :],
                                         start=(k == 0), stop=(k == KD - 1))
                    nc.vector.tensor_copy(h_sb[:, t, 0:H], h_ps[:])
                    scr = s1p.tile([128, H], f32, tag="scr")
                    nc.vector.tensor_tensor(out=scr[:], in0=h_sb[:, t, 0:H],
                                            in1=asrc_sb[:], op=ALU.mult)
                    nc.vector.tensor_reduce(out=h_sb[:, t, H:H + 1], in_=scr[:],
                                            axis=mybir.AxisListType.X, op=ALU.add)
                    scr2 = s1p.tile([128, H], f32, tag="scr")
                    nc.vector.tensor_tensor(out=scr2[:], in0=h_sb[:, t, 0:H],
                                            in1=adst_sb[:], op=ALU.mult)
                    nc.vector.tensor_reduce(out=edstloc[:, t:t + 1], in_=scr2[:],
                                            axis=mybir.AxisListType.X, op=ALU.add)

            # write local tables to DRAM
            ntf = NT if NLOC == NLOCP else NT - 1
            if ntf:
                nc.sync.dma_start(
                    bass.AP(hloc_d.tensor, 0, [[ROWW, 128], [128 * ROWW, ntf], [1, ROWW]]),
                    h_sb[:, 0:ntf, :])
            if NT > ntf:
                rows = NLOC - ntf * 128
                nc.sync.dma_start(
                    bass.AP(hloc_d.tensor, ntf * 128 * ROWW, [[ROWW, rows], [1, ROWW]]),
                    h_sb[:rows, NT - 1, :])

            nc.gpsimd.collective_compute(
                "AllGather", ALU.bypass, replica_groups=[list(range(NCORES))],
                ins=[hloc_d.opt()], outs=[hfull_d.opt()])

            _stage = int(os.environ.get("K_STAGE", "3"))
            if _stage == 1:
                # debug early-out: dump first A cols of h_sb
                for t in range(NT):
                    rows = min(128, NLOC - t * 128)
                    nc.sync.dma_start(out_t[t * 128: t * 128 + rows, :],
                                      h_sb[:rows, t, 0:A])

            s1big_cm.__exit__(None, None, None)

            # ================= stage 2: edge aggregation ===================
            h0T = bigp.tile([128, NLOCP], f32)
            if NLOC != NLOCP:
                nc.vector.memset(h0T[:, NLOC:NLOCP], 0.0)
            with tc.tile_pool(name="s2", bufs=2) as s2p, \
                 tc.tile_pool(name="s2s", bufs=4) as s2s, \
                 tc.tile_pool(name="s2ps", bufs=2, space="PSUM") as s2ps:
                boff = 0
                tlocs = []
                for c in range(NCH if _stage >= 2 else 0):
                    na, nb = NA[c], NB[c]
                    nbf = na + nb
                    # e_dst broadcast row for this chunk: edb[p, d] = e_dst[c*C+d]
                    nt_c = C // 128
                    tc0 = c * nt_c
                    tcols = min(nt_c, NT - tc0)
                    edB_ps = s2ps.tile([128, C], f32, tag="edB", bufs=1,
                                       name=f"edB_{c}")
                    for tt in range(tcols):
                        edT_ps = s2ps.tile([1, 128], f32, tag="edT", bufs=2,
                                           name=f"edT_{c}_{tt}")
                        nc.tensor.matmul(edT_ps[:], edstloc[:, tc0 + tt:tc0 + tt + 1],
                                         ident[:], start=True, stop=True)
                        edrow = s2p.tile([1, 128], f32, tag="edrow", bufs=4,
                                         name=f"edrow_{c}_{tt}")
                        nc.vector.tensor_copy(edrow[:], edT_ps[:])
                        nc.tensor.matmul(edB_ps[:, tt * 128:(tt + 1) * 128], onesr[:],
                                         edrow[:], start=True, stop=True)
                    edb = s2p.tile([128, C], f32, tag="edb", name=f"edb_{c}")
                    if tcols < nt_c:
                        nc.vector.memset(edb[:, tcols * 128:], 0.0)
                    nc.vector.tensor_copy(edb[:, 0:tcols * 128],
                                          edB_ps[:, 0:tcols * 128])

                    # t[e] = e_dst[dst_e] via onehot x edb, free-axis accumulated
                    tloc = bigp.tile([128, NBFmax], f32, name=f"tloc_{c}")
                    tlocs.append(tloc)
                    for b in range(nbf):
                        scrT = s2s.tile([128, C], f32, tag="scrT", bufs=8)
                        nc.vector.scalar_tensor_tensor(
                            out=scrT[:], in0=iota_sb[:],
                            scalar=dstr_sb[:, boff + b: boff + b + 1],
                            in1=edb[:], op0=ALU.is_equal, op1=ALU.mult,
                            accum_out=tloc[:, b:b + 1])
                    boff += nbf

                boff = 0
                for c in range(NCH if _stage >= 2 else 0):
                    na, nb = NA[c], NB[c]
                    nbf = na + nb
                    Cc = min(C, NLOC - c * C)
                    tloc = tlocs[c]
                    g_c = s2p.tile([128, NBFmax, ROWW], f32, tag="g", bufs=3)
                    co8 = boff * 8
                    if na:
                        nc.gpsimd.dma_gather(
                            g_c[:, 0:na, :], hfull_d[0:TH, :],
                            srci_sb[:, co8: co8 + na * 8],
                            na * 128, na * 128, ROWW, single_packet=False)
                    if nb:
                        nc.gpsimd.dma_gather(
                            g_c[:, na:nbf, :], hfull_d[TH:N, :],
                            srci_sb[:, co8 + na * 8: co8 + nbf * 8],
                            nb * 128, nb * 128, ROWW, single_packet=False)

                    eps = s2p.tile([128, NBFmax], f32, tag="eps")
                    nc.vector.tensor_tensor(
                        out=eps[:, 0:nbf], in0=g_c[:, 0:nbf, H:H + 1],
                        in1=tloc[:, 0:nbf], op=ALU.add)
                    lk = s2p.tile([128, NBFmax], f32, tag="lk")
                    nc.vector.tensor_scalar(
                        out=lk[:, 0:nbf], in0=eps[:, 0:nbf],
                        scalar1=NEG_SLOPE, scalar2=None, op0=ALU.mult)
                    nc.vector.tensor_tensor(
                        out=lk[:, 0:nbf], in0=lk[:, 0:nbf],
                        in1=eps[:, 0:nbf], op=ALU.max)
                    w = s2p.tile([128, NBFmax], f32, tag="w")
                    nc.scalar.activation(w[:, 0:nbf], lk[:, 0:nbf], AF.Exp)

                    agg_ps = s2ps.tile([128, C], f32, tag="agg")
                    den_ps = s2ps.tile([1, C], f32, tag="den")
                    for b in range(nbf):
                        S_b = s2s.tile([128, C], f32, tag="S", bufs=8)
                        nc.vector.tensor_scalar(
                            out=S_b[:], in0=iota_sb[:],
                            scalar1=dstr_sb[:, boff + b: boff + b + 1],
                            scalar2=w[:, b:b + 1],
                            op0=ALU.is_equal, op1=ALU.mult)
                        nc.tensor.matmul(agg_ps[:], g_c[:, b, 0:H], S_b[:],
                                         start=(b == 0), stop=(b == nbf - 1))
                        nc.tensor.matmul(den_ps[:], onesc[:], S_b[:],
                                         start=(b == 0), stop=(b == nbf - 1))
                    # flush chunk: h0T[:, c*C : c*C+Cc]
                    dmax = s2p.tile([1, C], f32, tag="dmax")
                    nc.vector.tensor_scalar(out=dmax[:], in0=den_ps[:],
                                            scalar1=1e-16, scalar2=None, op0=ALU.max)
                    rden = s2p.tile([1, C], f32, tag="rden")
                    nc.vector.reciprocal(rden[:], dmax[:])
                    rb_ps = s2ps.tile([128, C], f32, tag="rb", bufs=1)
                    nc.tensor.matmul(rb_ps[:], onesr[:], rden[:], start=True, stop=True)
                    agg_sb = s2p.tile([128, C], f32, tag="aggsb")
                    nc.scalar.copy(agg_sb[:], agg_ps[:])
                    gat = s2p.tile([128, C], f32, tag="gat")
                    nc.vector.tensor_tensor(out=gat[:], in0=agg_sb[:], in1=rb_ps[:],
                                            op=ALU.mult)
                    nc.scalar.activation(h0T[:, c * C: c * C + Cc], gat[:, 0:Cc],
                                         AF.Relu, bias=bgat_sb[:])
                    boff += nbf

            if _stage == 2:
                nc.vector.memset(h0T[:, 0:NLOC], 0.5)
                for t in range(NT):
                    rows = min(128, NLOC - t * 128)
                    nc.sync.dma_start(
                        bass.AP(out_t, t * 128 * A, [[1, A], [A, rows]]),
                        h0T[0:A, t * 128: t * 128 + rows])

            # ================= stage 3: BN0 + MLP + softmax ================
            h1T = bigp.tile([128, NLOCP], f32)
            if NLOC != NLOCP:
                nc.vector.memset(h1T[:, NLOC:NLOCP], 0.0)
            if _stage >= 3:
             with tc.tile_pool(name="s3", bufs=2) as s3p, \
                 tc.tile_pool(name="s3ps", bufs=2, space="PSUM") as s3ps:

                def bn_fold(hT, k, Wnext_sb, bnext_sb, M):
                    """training-BN over hT's NLOC cols, all-reduced; returns
                    (W', bias') folding the affine BN into the next layer."""
                    s1 = s3p.tile([128, 1], f32, tag="bn1")
                    nc.vector.tensor_reduce(out=s1[:], in_=hT[:, 0:NLOC],
                                            axis=mybir.AxisListType.X, op=ALU.add)
                    sq = s3p.tile([128, NLOCP], f32, tag="bnsq")
                    s2 = s3p.tile([128, 1], f32, tag="bn2t")
                    nc.scalar.activation(sq[:, 0:NLOC], hT[:, 0:NLOC], AF.Square,
                                         accum_out=s2[:])
                    bnio = s3p.tile([128, 2], f32, tag="bnio")
                    nc.vector.tensor_copy(bnio[:, 0:1], s1[:])
                    nc.vector.tensor_copy(bnio[:, 1:2], s2[:])
                    bn_in_d = bn_in_0 if k == 0 else bn_in_1
                    bn_out_d = bn_out_0 if k == 0 else bn_out_1
                    nc.sync.dma_start(bn_in_d[:], bnio[:])
                    if os.environ.get("K_NOAR"):
                        nc.sync.dma_start(bn_out_d[:], bn_in_d[:])
                        # fake the x8: scale by 8 below via 1/N per-core count
                    else:
                        nc.gpsimd.collective_compute(
                            "AllReduce", ALU.add, replica_groups=[list(range(NCORES))],
                            ins=[bn_in_d.opt()], outs=[bn_out_d.opt()])
                    bnst = s3p.tile([128, 2], f32, tag="bnst")
                    nc.sync.dma_start(bnst[:], bn_out_d[:])
                    mu = s3p.tile([128, 1], f32, tag="mu")
                    nc.vector.tensor_scalar(out=mu[:], in0=bnst[:, 0:1],
                                            scalar1=1.0 / N, scalar2=None, op0=ALU.mult)
                    var = s3p.tile([128, 1], f32, tag="var")
                    # var = E[x^2] - mu^2 + EPS
                    nc.vector.tensor_tensor(out=var[:], in0=mu[:], in1=mu[:], op=ALU.mult)
                    nc.vector.tensor_scalar(out=var[:], in0=var[:], scalar1=-1.0,
                                            scalar2=None, op0=ALU.mult)
                    nc.vector.scalar_tensor_tensor(
                        out=var[:], in0=bnst[:, 1:2], scalar=1.0 / N, in1=var[:],
                        op0=ALU.mult, op1=ALU.add)
                    nc.vector.tensor_scalar(out=var[:], in0=var[:], scalar1=EPS,
                                            scalar2=None, op0=ALU.add)
                    rs = s3p.tile([128, 1], f32, tag="rs")
                    nc.vector.reciprocal(rs[:], var[:])
                    nc.scalar.sqrt(rs[:], rs[:])
                    bnp = bn0_sb if k == 0 else bn2_sb
                    sc = s3p.tile([128, 1], f32, tag="sc")
                    nc.vector.tensor_tensor(out=sc[:], in0=rs[:], in1=bnp[:, 0:1],
                                            op=ALU.mult)
                    u = s3p.tile([128, 1], f32, tag="u")
                    nc.vector.tensor_tensor(out=u[:], in0=mu[:], in1=sc[:], op=ALU.mult)
                    nc.vector.tensor_sub(u[:], bnp[:, 1:2], u[:])
                    Wp = s3p.tile([128, M], f32, tag="wp" + str(k))
                    nc.vector.tensor_scalar(out=Wp[:], in0=Wnext_sb[:], scalar1=sc[:],
                                            scalar2=None, op0=ALU.mult)
                    brow_ps = s3ps.tile([1, M], f32, tag="brow", bufs=1)
                    nc.tensor.matmul(brow_ps[:], u[:], Wnext_sb[:], start=True, stop=True)
                    brow_sb = s3p.tile([1, M], f32, tag="brsb")
                    nc.vector.tensor_copy(brow_sb[:], brow_ps[:])
                    bcol_ps = s3ps.tile([M, 1], f32, tag="bcol", bufs=1)
                    nc.tensor.transpose(bcol_ps[:], brow_sb[:], ident[0:1, 0:1])
                    bp = s3p.tile([M, 1], f32, tag="bp" + str(k))
                    nc.vector.tensor_tensor(out=bp[:], in0=bcol_ps[:], in1=bnext_sb[:],
                                            op=ALU.add)
                    return Wp, bp

                _s3 = os.environ.get("K_S3", "")
                if _s3 == "mlp":
                    W1p, b1p = W1_sb, b1_sb
                else:
                    W1p, b1p = bn_fold(h0T, 0, W1_sb, b1_sb, H)
                for s in range(0, NLOC, 512):
                    ln = min(512, NLOC - s)
                    ps = s3ps.tile([128, 512], f32, tag="mlp")
                    nc.tensor.matmul(ps[:, 0:ln], W1p[:], h0T[:, s:s + ln],
                                     start=True, stop=True)
                    nc.scalar.activation(h1T[:, s:s + ln], ps[:, 0:ln], AF.Relu,
                                         bias=b1p[:])
                h2T = h0T  # reuse buffer
                for s in range(0, NLOC, 512):
                    ln = min(512, NLOC - s)
                    ps = s3ps.tile([128, 512], f32, tag="mlp")
                    nc.tensor.matmul(ps[:, 0:ln], W2_sb[:], h1T[:, s:s + ln],
                                     start=True, stop=True)
                    nc.scalar.activation(h2T[:, s:s + ln], ps[:, 0:ln], AF.Relu,
                                         bias=b2_sb[:])
                if _s3 == "mlp":
                    W3p, b3p = W3_sb, b3_sb
                else:
                    W3p, b3p = bn_fold(h2T, 1, W3_sb, b3_sb, A)
                actT = bigp.tile([A, NLOCP], f32)
                nc.vector.memset(actT[:], 0.0)
                for s in range(0, NLOC, 512):
                    ln = min(512, NLOC - s)
                    ps = s3ps.tile([A, 512], f32, tag="mlp")
                    nc.tensor.matmul(ps[:, 0:ln], W3p[:], h2T[:, s:s + ln],
                                     start=True, stop=True)
                    nc.vector.tensor_scalar(out=actT[0:A, s:s + ln], in0=ps[:, 0:ln],
                                            scalar1=b3p[:], scalar2=None, op0=ALU.add)
                if os.environ.get("K_NOSOFT"):
                    for t in range(NT):
                        rows = min(128, NLOC - t * 128)
                        nc.sync.dma_start(
                            bass.AP(out_t, t * 128 * A, [[1, A], [A, rows]]),
                            actT[0:A, t * 128: t * 128 + rows])
                # row softmax + output
                for t in range(NT if not os.environ.get("K_NOSOFT") else 0):
                    rows = min(128, NLOC - t * 128)
                    a_sb = s3p.tile([128, A], f32, tag="asb")
                    assert A == 32
                    for sub in range(4):
                        nc.vector.transpose(
                            a_sb[32 * sub:32 * sub + 32, 0:A],
                            actT[0:A, t * 128 + 32 * sub: t * 128 + 32 * sub + 32])
                    if os.environ.get("K_SM") == "t":
                        nc.sync.dma_start(out_t[t * 128: t * 128 + rows, :],
                                          a_sb[:rows, :])
                        continue
                    nmax = s3p.tile([128, 1], f32, tag="nmax")
                    nc.vector.tensor_reduce(out=nmax[:], in_=a_sb[:],
                                            axis=mybir.AxisListType.X, op=ALU.max)
                    nc.vector.tensor_scalar(out=nmax[:], in0=nmax[:], scalar1=-1.0,
                                            scalar2=None, op0=ALU.mult)
                    e_sb = s3p.tile([128, A], f32, tag="esb")
                    nc.scalar.activation(e_sb[:], a_sb[:], AF.Exp, bias=nmax[:])
                    ssum = s3p.tile([128, 1], f32, tag="ssum")
                    nc.vector.tensor_reduce(out=ssum[:], in_=e_sb[:],
                                            axis=mybir.AxisListType.X, op=ALU.add)
                    rsum = s3p.tile([128, 1], f32, tag="rsum")
                    nc.vector.reciprocal(rsum[:], ssum[:])
                    o_sb = s3p.tile([128, A], f32, tag="osb")
                    nc.vector.tensor_scalar(out=o_sb[:], in0=e_sb[:], scalar1=rsum[:],
                                            scalar2=None, op0=ALU.mult)
                    nc.sync.dma_start(out_t[t * 128: t * 128 + rows, :], o_sb[:rows, :])

    nc.compile()
    return nc


# --------------------------------------------------------------------------
# public entry point
# --------------------------------------------------------------------------

def run(inputs, trace=False):
    global last_results
    x = np.asarray(inputs["x"], np.float32)
    edge_index = np.asarray(inputs["edge_index"])
    N, D = x.shape
    H = np.asarray(inputs["W"]).shape[1]
    A = np.asarray(inputs["W3"]).shape[1]
    assert N % NCORES == 0
    NLOC = N // NCORES
    TH = 25000 if N > 32000 else N
    C = min(CHUNK_C, NLOC)

    per_core, NA, NB = _prep_edges(edge_index, N, NLOC, TH, C)
    TOTB = sum(NA) + sum(NB)

    key = (N, D, H, A, NLOC, TH, C, tuple(NA), tuple(NB))
    if _cache.get("key") != key:
        _cache["nc"] = _build_nc(N, D, H, A, NLOC, TH, C, NA, NB)
        _cache["key"] = key
    nc = _cache["nc"]

    g = lambda k: np.ascontiguousarray(np.asarray(inputs[k], np.float32))
    common = {
        "W": g("W"),
        "asrc_b": np.tile(g("a_src")[None, :], (128, 1)),
        "adst_b": np.tile(g("a_dst")[None, :], (128, 1)),
        "b_gat": g("b_gat").reshape(H, 1),
        "bn0p": np.stack([g("g0"), g("beta0")], 1),
        "bn2p": np.stack([g("g2"), g("beta2")], 1),
        "W1": g("W1"), "b1": g("b1").reshape(H, 1),
        "W2": g("W2"), "b2": g("b2").reshape(H, 1),
        "W3": g("W3"), "b3": g("b3").reshape(A, 1),
        "ident": np.eye(128, dtype=np.float32),
        "iota_b": np.tile(np.arange(C, dtype=np.float32)[None, :], (128, 1)),
        "ones_col": np.ones((128, 1), np.float32),
        "ones_row": np.ones((1, 128), np.float32),
    }
    in_maps = []
    for i in range(NCORES):
        m = dict(common)
        m["x_shard"] = np.ascontiguousarray(x[i * NLOC:(i + 1) * NLOC])
        m["src_idx"] = per_core[i]["src_idx"]
        m["dst_rel"] = per_core[i]["dst_rel"]
        in_maps.append(m)

    last_results = run_bass_kernel_spmd(nc, in_maps, list(range(NCORES)),
                                        trace=trace)
    out = np.concatenate([last_results.results[i]["out"] for i in range(NCORES)], 0)
    return out


def kernel(**inputs) -> np.ndarray:
    return run(inputs, trace=False)



# revision 4
# speedup vs baseline: 1.4904x; 1.4904x over previous
"""Distributed Trainium2 Bass kernel for the GAT-Actor (gnn_message_passing).

Strategy (8 NeuronCores, 1-D node partition):
  - nodes sharded contiguously: core i owns rows [i*NLOC, (i+1)*NLOC)
  - edges assigned to the core owning their DESTINATION node
  - stage 1 (f32): h = x_shard @ W (x pre-transposed on host); per node a
    768B f32 table row [h(128) | 1.0 | e_src | pad]; rows are written in two
    pieces (locals < 3200 / >= 3200) and AllGathered piece-wise so the edge
    stage can start after the first collective.
  - stage 2: edges sorted by (dst-chunk, src-piece) into 128-edge blocks;
    dma_gather pulls 768B rows.  The dst-scatter onehot P[e, d] per block is
    HOST-precomputed (0/1 bf16, pad rows zero) and streamed from HBM - no
    on-device onehot builds.  Per block:
      tloc (e_dst per edge) = DVE stt accum of P * broadcast(e_dst row),
      S = ACT copy-scale of P by w (per-partition scalar, f32 out),
      agg[d, 0:130] += S^T [h | 1 | e_src]  (TensorE; col 128 = softmax
      denominator - ones column fused, no separate den matmul).
    Chunk tail: scale rows by 1/den, PE-transpose to feature-major,
    bias+relu -> h0T f32.
  - stage 3 (f32): BN stats via 1KB AllReduce folded into rescaled fc
    weights; fc1/fc2/fc3 on TensorE; row softmax; [NLOC, 32] shards
    concatenated on host.
"""

import os
import sys

for _p in ("/opt/trn_rl_repo", "/root/.axon_site/_ro/trn_rl_repo"):
    if os.path.isdir(_p) and _p not in sys.path:
        sys.path.insert(0, _p)

import numpy as np
import ml_dtypes

from concourse import bass, bacc, tile, mybir
from concourse.bass_utils import run_bass_kernel_spmd

f32 = mybir.dt.float32
bf16 = mybir.dt.bfloat16
i16 = mybir.dt.int16
AF = mybir.ActivationFunctionType
ALU = mybir.AluOpType

NCORES = 8
C = 128                # dst-chunk width
NEG_SLOPE = 0.2
EPS = 1e-5
PL = 3200              # piece boundary in local rows (25 tiles of 128)
G_CH = 2               # chunks per gather group

_cache = {}
last_results = None


# --------------------------------------------------------------------------
# host-side edge preprocessing
# --------------------------------------------------------------------------

def _wrap_idx(idx):
    """int16 index stream -> [128, len/16] wrapped+replicated for dma_gather."""
    idx = np.asarray(idx, np.int16)
    m = idx.shape[0]
    assert m % 16 == 0
    arr = idx.reshape(m // 16, 16).T
    return np.ascontiguousarray(np.tile(arr, (8, 1)))


def _prep_edges(edge_index, N, NLOC):
    """Sort edges per dst-core by (dst-chunk, src-piece); pad each
    (chunk, piece) to 128-edge blocks shared across cores.  Returns the
    per-core index streams + onehot P tiles and the shared block layout.
    """
    src = np.asarray(edge_index[0], np.int64)
    dst = np.asarray(edge_index[1], np.int64)
    NCH = -(-NLOC // C)
    PB = NLOC - PL

    cores = []
    counts = np.zeros((NCORES, NCH, 2), np.int64)
    for i in range(NCORES):
        sel = (dst // NLOC) == i
        s = src[sel]
        d = dst[sel] - i * NLOC
        ch = d // C
        cs = s // NLOC
        loc = s % NLOC
        hf = (loc >= PL).astype(np.int64)
        idx16 = np.where(hf == 0, cs * PL + loc, cs * PB + (loc - PL))
        order = np.lexsort((hf, ch))
        s_i, d_i, ch_i, hf_i = idx16[order], d[order], ch[order], hf[order]
        for c in range(NCH):
            m = ch_i == c
            counts[i, c, 0] = np.count_nonzero(m & (hf_i == 0))
            counts[i, c, 1] = np.count_nonzero(m & (hf_i == 1))
        cores.append((s_i, d_i, ch_i, hf_i))

    NA = [int(-(-counts[:, c, 0].max() // 128)) for c in range(NCH)]
    NB = [int(-(-counts[:, c, 1].max() // 128)) for c in range(NCH)]

    groups = [list(range(g, min(g + G_CH, NCH))) for g in range(0, NCH, G_CH)]

    blk_of = {}
    goff = 0
    ginfo = []
    for chunks in groups:
        nA = sum(NA[c] for c in chunks)
        nB = sum(NB[c] for c in chunks)
        off = goff
        for c in chunks:
            blk_of[(c, 0)] = off
            off += NA[c]
        for c in chunks:
            blk_of[(c, 1)] = off
            off += NB[c]
        ginfo.append((chunks, goff, nA, nB))
        goff += nA + nB
    TOTB = goff
    TOTE = TOTB * 128

    dr = np.arange(C, dtype=np.int64)
    per_core = []
    for i in range(NCORES):
        s_i, d_i, ch_i, hf_i = cores[i]
        src_idx = np.zeros(TOTE, np.int16)
        dst_rel = np.full(TOTE, -1, np.int64)
        ptr = 0
        for c in range(NCH):
            for h in (0, 1):
                cnt = int(counts[i, c, h])
                sl = slice(ptr, ptr + cnt)
                ptr += cnt
                pos = blk_of[(c, h)] * 128
                if cnt:
                    src_idx[pos:pos + cnt] = s_i[sl].astype(np.int16)
                    dst_rel[pos:pos + cnt] = d_i[sl] - c * C
        assert ptr == len(s_i)
        # P tiles: [TOTB*128, 128] bf16, row b*128+e = onehot(dst_rel)
        drel = dst_rel.reshape(TOTB, 128)
        P = (drel[:, :, None] == dr[None, None, :]).astype(ml_dtypes.bfloat16)
        per_core.append({
            "src_idx": _wrap_idx(src_idx),
            "P": np.ascontiguousarray(P.reshape(TOTB * 128, 128)),
        })
    return per_core, NA, NB, ginfo, blk_of, TOTB


# --------------------------------------------------------------------------
# device graph
# --------------------------------------------------------------------------

def _build_nc(N, D, H, A, NLOC, NA, NB, ginfo, blk_of, TOTB):
    KD = D // 128
    NT = -(-NLOC // 128)
    NLOCP = NT * 128
    NCH = len(NA)
    PB = NLOC - PL
    RA = NCORES * PL
    RB = NCORES * PB
    ROWW = 192                # f32 elems per table row (768B)
    MRW = 130                 # meaningful row width: h(128) | 1 | e_src

    nc = bacc.Bacc("TRN2", num_devices=NCORES)

    xT_in = nc.dram_tensor("xT_shard", [D, NLOC], f32, kind="ExternalInput")
    W_in = nc.dram_tensor("W", [D, H], f32, kind="ExternalInput")
    asrcb = nc.dram_tensor("asrc_b", [128, H], f32, kind="ExternalInput")
    adstb = nc.dram_tensor("adst_b", [128, H], f32, kind="ExternalInput")
    bgat = nc.dram_tensor("b_gat", [H, 1], f32, kind="ExternalInput")
    bn0p = nc.dram_tensor("bn0p", [H, 2], f32, kind="ExternalInput")
    bn2p = nc.dram_tensor("bn2p", [H, 2], f32, kind="ExternalInput")
    W1_in = nc.dram_tensor("W1", [H, H], f32, kind="ExternalInput")
    b1_in = nc.dram_tensor("b1", [H, 1], f32, kind="ExternalInput")
    W2_in = nc.dram_tensor("W2", [H, H], f32, kind="ExternalInput")
    b2_in = nc.dram_tensor("b2", [H, 1], f32, kind="ExternalInput")
    W3_in = nc.dram_tensor("W3", [H, A], f32, kind="ExternalInput")
    b3_in = nc.dram_tensor("b3", [A, 1], f32, kind="ExternalInput")
    ident_in = nc.dram_tensor("ident", [128, 128], f32, kind="ExternalInput")
    onesrow_in = nc.dram_tensor("ones_row", [1, 128], f32, kind="ExternalInput")
    srci_in = nc.dram_tensor("src_idx", [128, TOTB * 8], i16, kind="ExternalInput")
    P_in = nc.dram_tensor("P", [TOTB * 128, 128], bf16, kind="ExternalInput")

    out_t = nc.dram_tensor("out", [NLOC, A], f32, kind="ExternalOutput")

    with tile.TileContext(nc) as tc:
        with tc.tile_pool(name="const", bufs=1) as cp, \
             tc.tile_pool(name="dram", bufs=1, space="DRAM") as dram, \
             tc.tile_pool(name="big", bufs=1) as bigp:

            srci_sb = bigp.tile([128, TOTB * 8], i16)
            nc.sync.dma_start(srci_sb[:], srci_in[:])
            W_sb = cp.tile([128, KD, H], f32)
            nc.sync.dma_start(W_sb[:], bass.AP(W_in, 0, [[H, 128], [128 * H, KD], [1, H]]))
            ident = cp.tile([128, 128], f32)
            nc.sync.dma_start(ident[:], ident_in[:])
            asrc_sb = cp.tile([128, H], f32)
            nc.sync.dma_start(asrc_sb[:], asrcb[:])
            adst_sb = cp.tile([128, H], f32)
            nc.sync.dma_start(adst_sb[:], adstb[:])
            bgat_sb = cp.tile([H, 1], f32)
            nc.sync.dma_start(bgat_sb[:], bgat[:])
            bn0_sb = cp.tile([H, 2], f32)
            nc.sync.dma_start(bn0_sb[:], bn0p[:])
            bn2_sb = cp.tile([H, 2], f32)
            nc.sync.dma_start(bn2_sb[:], bn2p[:])
            W1_sb = cp.tile([H, H], f32)
            nc.sync.dma_start(W1_sb[:], W1_in[:])
            b1_sb = cp.tile([H, 1], f32)
            nc.sync.dma_start(b1_sb[:], b1_in[:])
            W2_sb = cp.tile([H, H], f32)
            nc.sync.dma_start(W2_sb[:], W2_in[:])
            b2_sb = cp.tile([H, 1], f32)
            nc.sync.dma_start(b2_sb[:], b2_in[:])
            W3_sb = cp.tile([H, A], f32)
            nc.sync.dma_start(W3_sb[:], W3_in[:])
            b3_sb = cp.tile([A, 1], f32)
            nc.sync.dma_start(b3_sb[:], b3_in[:])
            onesr = cp.tile([1, 128], f32)
            nc.sync.dma_start(onesr[:], onesrow_in[:])

            hlocA = dram.tile([PL, ROWW], f32)
            hlocB = dram.tile([PB, ROWW], f32)
            hfullA = dram.tile([RA, ROWW], f32, addr_space="Shared")
            hfullB = dram.tile([RB, ROWW], f32, addr_space="Shared")
            bn_in_0 = dram.tile([H, 2], f32)
            bn_out_0 = dram.tile([H, 2], f32, addr_space="Shared")
            bn_in_1 = dram.tile([H, 2], f32)
            bn_out_1 = dram.tile([H, 2], f32, addr_space="Shared")

            edstloc = bigp.tile([128, NT], f32)

            # ================= stage 1: h rows + e_src/e_dst ================
            with tc.tile_pool(name="s1", bufs=3) as s1p, \
                 tc.tile_pool(name="s1ps", bufs=2, space="PSUM") as s1ps:
                for t in range(NT):
                    rows = min(128, NLOC - t * 128)
                    xT_t = s1p.tile([128, KD, 128], f32, tag="xt")
                    for k in range(KD):
                        nc.sync.dma_start(
                            xT_t[:, k, 0:rows],
                            xT_in[k * 128:(k + 1) * 128,
                                  t * 128:t * 128 + rows])
                    h_ps = s1ps.tile([128, H], f32, tag="hps")
                    for k in range(KD):
                        nc.tensor.matmul(h_ps[:], xT_t[:, k, :], W_sb[:, k, :],
                                         start=(k == 0), stop=(k == KD - 1))
                    h_row = s1p.tile([128, MRW], f32, tag="hrow")
                    nc.vector.tensor_copy(h_row[:, 0:H], h_ps[:])
                    nc.vector.memset(h_row[:, H:H + 1], 1.0)
                    scr = s1p.tile([128, H], f32, tag="scr")
                    nc.vector.scalar_tensor_tensor(
                        out=scr[:], in0=h_ps[:], scalar=1.0, in1=asrc_sb[:],
                        op0=ALU.mult, op1=ALU.mult,
                        accum_out=h_row[:, MRW - 1:MRW])
                    scr2 = s1p.tile([128, H], f32, tag="scr2")
                    nc.vector.scalar_tensor_tensor(
                        out=scr2[:], in0=h_ps[:], scalar=1.0, in1=adst_sb[:],
                        op0=ALU.mult, op1=ALU.mult,
                        accum_out=edstloc[:, t:t + 1])
                    if t < 25:
                        nc.sync.dma_start(
                            bass.AP(hlocA.tensor, t * 128 * ROWW,
                                    [[ROWW, rows], [1, MRW]]),
                            h_row[:rows, :])
                    else:
                        r0 = (t - 25) * 128
                        nc.sync.dma_start(
                            bass.AP(hlocB.tensor, r0 * ROWW,
                                    [[ROWW, rows], [1, MRW]]),
                            h_row[:rows, :])

            nc.gpsimd.collective_compute(
                "AllGather", ALU.bypass, replica_groups=[list(range(NCORES))],
                ins=[hlocA.opt()], outs=[hfullA.opt()])
            nc.gpsimd.collective_compute(
                "AllGather", ALU.bypass, replica_groups=[list(range(NCORES))],
                ins=[hlocB.opt()], outs=[hfullB.opt()])

            # ================= stage 2: edge aggregation ===================
            h0T = bigp.tile([128, NLOCP], f32)
            if NLOC != NLOCP:
                nc.vector.memset(h0T[:, NLOC:NLOCP], 0.0)
            with tc.tile_pool(name="s2", bufs=2) as s2p, \
                 tc.tile_pool(name="s2s", bufs=4) as s2s, \
                 tc.tile_pool(name="s2ps", bufs=2, space="PSUM") as s2ps:
                for chunks, goff, nAg, nBg in ginfo:
                    nblk = nAg + nBg
                    g_t = s2p.tile([128, nblk, ROWW], f32, tag="g")
                    if nAg:
                        nc.gpsimd.dma_gather(
                            g_t[:, 0:nAg, :], hfullA[:],
                            srci_sb[:, goff * 8: (goff + nAg) * 8],
                            nAg * 128, nAg * 128, ROWW, single_packet=False)
                    if nBg:
                        nc.gpsimd.dma_gather(
                            g_t[:, nAg:nblk, :], hfullB[:],
                            srci_sb[:, (goff + nAg) * 8: (goff + nblk) * 8],
                            nBg * 128, nBg * 128, ROWW, single_packet=False)
                    P_t = s2p.tile([128, nblk, 128], bf16, tag="P")
                    nc.sync.dma_start(
                        P_t[:],
                        bass.AP(P_in, goff * 128 * 128,
                                [[128, 128], [128 * 128, nblk], [1, 128]]))

                    for c in chunks:
                        na, nb = NA[c], NB[c]
                        nbf = na + nb
                        aoff = blk_of[(c, 0)] - goff
                        boff = blk_of[(c, 1)] - goff
                        Cc = min(C, NLOC - c * C)
                        blist = list(range(aoff, aoff + na)) + \
                                list(range(boff, boff + nb))

                        # e_dst broadcast row for this chunk
                        edT_ps = s2ps.tile([1, 128], f32, tag="edT", bufs=2)
                        nc.tensor.matmul(edT_ps[:], edstloc[:, c:c + 1],
                                         ident[:], start=True, stop=True)
                        edrow = s2p.tile([1, 128], f32, tag="edrow", bufs=2)
                        nc.vector.tensor_copy(edrow[:], edT_ps[:])
                        edB_ps = s2ps.tile([128, 128], f32, tag="edB", bufs=2)
                        nc.tensor.matmul(edB_ps[:], onesr[:], edrow[:],
                                         start=True, stop=True)
                        edb = s2p.tile([128, 128], f32, tag="edb", bufs=2)
                        nc.vector.tensor_copy(edb[:], edB_ps[:])

                        # per-edge e_dst: tloc[e] = sum_d P[e,d] * edb[.,d]
                        tlocv = s2p.tile([128, nbf], f32, tag="tloc", bufs=2)
                        for j, b in enumerate(blist):
                            scrT = s2s.tile([128, C], f32, tag="scrT", bufs=8)
                            nc.vector.scalar_tensor_tensor(
                                out=scrT[:], in0=P_t[:, b, :], scalar=1.0,
                                in1=edb[:], op0=ALU.mult, op1=ALU.mult,
                                accum_out=tlocv[:, j:j + 1])

                        # w = exp(leaky(e_src + e_dst))
                        eps_t = s2p.tile([128, nbf], f32, tag="eps", bufs=2)
                        nc.vector.tensor_tensor(
                            out=eps_t[:, 0:na],
                            in0=g_t[:, aoff:aoff + na, MRW - 1],
                            in1=tlocv[:, 0:na], op=ALU.add)
                        nc.vector.tensor_tensor(
                            out=eps_t[:, na:nbf],
                            in0=g_t[:, boff:boff + nb, MRW - 1],
                            in1=tlocv[:, na:nbf], op=ALU.add)
                        lk = s2p.tile([128, nbf], f32, tag="lk", bufs=2)
                        nc.vector.scalar_tensor_tensor(
                            out=lk[:], in0=eps_t[:], scalar=NEG_SLOPE,
                            in1=eps_t[:], op0=ALU.mult, op1=ALU.max)
                        w_t = s2p.tile([128, nbf], f32, tag="w", bufs=2)
                        nc.scalar.activation(w_t[:], lk[:], AF.Exp)

                        # agg[d, 0:130] += S^T [h | 1 | e_src],  S = P * w
                        agg_ps = s2ps.tile([128, MRW], f32, tag="agg", bufs=2)
                        for j, b in enumerate(blist):
                            S_b = s2s.tile([128, C], f32, tag="S", bufs=8)
                            nc.scalar.activation(S_b[:], P_t[:, b, :], AF.Copy,
                                                 scale=w_t[:, j:j + 1])
                            nc.tensor.matmul(agg_ps[:], S_b[:],
                                             g_t[:, b, 0:MRW],
                                             start=(j == 0), stop=(j == nbf - 1))

                        den = s2p.tile([128, 1], f32, tag="den", bufs=2)
                        nc.vector.tensor_scalar(
                            out=den[:], in0=agg_ps[:, H:H + 1],
                            scalar1=1e-16, scalar2=None, op0=ALU.max)
                        rden = s2p.tile([128, 1], f32, tag="rden", bufs=2)
                        nc.vector.reciprocal(rden[:], den[:])
                        h0n = s2p.tile([128, 128], f32, tag="h0n", bufs=2)
                        nc.vector.tensor_scalar(
                            out=h0n[:], in0=agg_ps[:, 0:H],
                            scalar1=rden[:], scalar2=None, op0=ALU.mult)
                        tr_ps = s2ps.tile([128, 128], f32, tag="tr", bufs=2)
                        nc.tensor.transpose(tr_ps[:], h0n[:], ident[:])
                        nc.vector.tensor_scalar(
                            out=h0T[:, c * C: c * C + Cc], in0=tr_ps[:, 0:Cc],
                            scalar1=bgat_sb[:], scalar2=0.0,
                            op0=ALU.add, op1=ALU.max)

            # ================= stage 3: BN0 + MLP + softmax ================
            with tc.tile_pool(name="s3", bufs=2) as s3p, \
                 tc.tile_pool(name="s3ps", bufs=2, space="PSUM") as s3ps:

                def bn_fold(hT, k, Wnext_sb, bnext_sb, M):
                    s1 = s3p.tile([128, 1], f32, tag="bn1")
                    nc.vector.tensor_reduce(out=s1[:], in_=hT[:, 0:NLOC],
                                            axis=mybir.AxisListType.X, op=ALU.add)
                    nsq = -(-NLOC // 512)
                    s2cols = s3p.tile([128, nsq], f32, tag="bnsq" + str(k))
                    for si in range(nsq):
                        s0 = si * 512
                        ln = min(512, NLOC - s0)
                        sq = s3p.tile([128, 512], f32, tag="sqscr", bufs=2)
                        nc.scalar.activation(sq[:, 0:ln], hT[:, s0:s0 + ln],
                                             AF.Square,
                                             accum_out=s2cols[:, si:si + 1])
                    s2 = s3p.tile([128, 1], f32, tag="bn2t")
                    nc.vector.tensor_reduce(out=s2[:], in_=s2cols[:],
                                            axis=mybir.AxisListType.X, op=ALU.add)
                    bnio = s3p.tile([128, 2], f32, tag="bnio")
                    nc.vector.tensor_copy(bnio[:, 0:1], s1[:])
                    nc.vector.tensor_copy(bnio[:, 1:2], s2[:])
                    bn_in_d = bn_in_0 if k == 0 else bn_in_1
                    bn_out_d = bn_out_0 if k == 0 else bn_out_1
                    nc.sync.dma_start(bn_in_d[:], bnio[:])
                    nc.gpsimd.collective_compute(
                        "AllReduce", ALU.add, replica_groups=[list(range(NCORES))],
                        ins=[bn_in_d.opt()], outs=[bn_out_d.opt()])
                    bnst = s3p.tile([128, 2], f32, tag="bnst")
                    nc.sync.dma_start(bnst[:], bn_out_d[:])
                    mu = s3p.tile([128, 1], f32, tag="mu")
                    nc.vector.tensor_scalar(out=mu[:], in0=bnst[:, 0:1],
                                            scalar1=1.0 / N, scalar2=None,
                                            op0=ALU.mult)
                    var = s3p.tile([128, 1], f32, tag="var")
                    nc.vector.tensor_tensor(out=var[:], in0=mu[:], in1=mu[:],
                                            op=ALU.mult)
                    nc.vector.tensor_scalar(out=var[:], in0=var[:], scalar1=-1.0,
                                            scalar2=None, op0=ALU.mult)
                    nc.vector.scalar_tensor_tensor(
                        out=var[:], in0=bnst[:, 1:2], scalar=1.0 / N, in1=var[:],
                        op0=ALU.mult, op1=ALU.add)
                    nc.vector.tensor_scalar(out=var[:], in0=var[:], scalar1=EPS,
                                            scalar2=None, op0=ALU.add)
                    rs = s3p.tile([128, 1], f32, tag="rs")
                    nc.vector.reciprocal(rs[:], var[:])
                    nc.scalar.sqrt(rs[:], rs[:])
                    bnp = bn0_sb if k == 0 else bn2_sb
                    sc = s3p.tile([128, 1], f32, tag="sc")
                    nc.vector.tensor_tensor(out=sc[:], in0=rs[:], in1=bnp[:, 0:1],
                                            op=ALU.mult)
                    u = s3p.tile([128, 1], f32, tag="u")
                    nc.vector.tensor_tensor(out=u[:], in0=mu[:], in1=sc[:],
                                            op=ALU.mult)
                    nc.vector.tensor_sub(u[:], bnp[:, 1:2], u[:])
                    Wp = s3p.tile([128, M], f32, tag="wp" + str(k))
                    nc.vector.tensor_scalar(out=Wp[:], in0=Wnext_sb[:],
                                            scalar1=sc[:], scalar2=None,
                                            op0=ALU.mult)
                    brow_ps = s3ps.tile([1, M], f32, tag="brow", bufs=1)
                    nc.tensor.matmul(brow_ps[:], u[:], Wnext_sb[:],
                                     start=True, stop=True)
                    brow_sb = s3p.tile([1, M], f32, tag="brsb")
                    nc.vector.tensor_copy(brow_sb[:], brow_ps[:])
                    bcol_ps = s3ps.tile([M, 1], f32, tag="bcol", bufs=1)
                    nc.tensor.transpose(bcol_ps[:], brow_sb[:], ident[0:1, 0:1])
                    bp = s3p.tile([M, 1], f32, tag="bp" + str(k))
                    nc.vector.tensor_tensor(out=bp[:], in0=bcol_ps[:],
                                            in1=bnext_sb[:], op=ALU.add)
                    return Wp, bp

                h1T = bigp.tile([128, NLOCP], f32)
                W1p, b1p = bn_fold(h0T, 0, W1_sb, b1_sb, H)
                for s in range(0, NLOC, 512):
                    ln = min(512, NLOC - s)
                    ps = s3ps.tile([128, 512], f32, tag="mlp")
                    nc.tensor.matmul(ps[:, 0:ln], W1p[:], h0T[:, s:s + ln],
                                     start=True, stop=True)
                    nc.scalar.activation(h1T[:, s:s + ln], ps[:, 0:ln], AF.Relu,
                                         bias=b1p[:])
                h2T = h0T  # overwrite in place
                for s in range(0, NLOC, 512):
                    ln = min(512, NLOC - s)
                    ps = s3ps.tile([128, 512], f32, tag="mlp")
                    nc.tensor.matmul(ps[:, 0:ln], W2_sb[:], h1T[:, s:s + ln],
                                     start=True, stop=True)
                    nc.scalar.activation(h2T[:, s:s + ln], ps[:, 0:ln], AF.Relu,
                                         bias=b2_sb[:])
                W3p, b3p = bn_fold(h2T, 1, W3_sb, b3_sb, A)
                actT = bigp.tile([A, NLOCP], f32)
                for s in range(0, NLOC, 512):
                    ln = min(512, NLOC - s)
                    ps = s3ps.tile([A, 512], f32, tag="mlp3")
                    nc.tensor.matmul(ps[:, 0:ln], W3p[:], h2T[:, s:s + ln],
                                     start=True, stop=True)
                    nc.vector.tensor_scalar(out=actT[0:A, s:s + ln],
                                            in0=ps[:, 0:ln],
                                            scalar1=b3p[:], scalar2=None,
                                            op0=ALU.add)
                for t in range(NT):
                    rows = min(128, NLOC - t * 128)
                    a_sb = s3p.tile([128, A], f32, tag="asb")
                    for sub in range(4):
                        nc.vector.transpose(
                            a_sb[32 * sub:32 * sub + 32, 0:A],
                            actT[0:A, t * 128 + 32 * sub: t * 128 + 32 * sub + 32])
                    nmax = s3p.tile([128, 1], f32, tag="nmax")
                    nc.vector.tensor_reduce(out=nmax[:], in_=a_sb[:],
                                            axis=mybir.AxisListType.X, op=ALU.max)
                    nc.vector.tensor_scalar(out=nmax[:], in0=nmax[:],
                                            scalar1=-1.0, scalar2=None,
                                            op0=ALU.mult)
                    e_sb = s3p.tile([128, A], f32, tag="esb")
                    nc.scalar.activation(e_sb[:], a_sb[:], AF.Exp, bias=nmax[:])
                    ssum = s3p.tile([128, 1], f32, tag="ssum")
                    nc.vector.tensor_reduce(out=ssum[:], in_=e_sb[:],
                                            axis=mybir.AxisListType.X, op=ALU.add)
                    rsum = s3p.tile([128, 1], f32, tag="rsum")
                    nc.vector.reciprocal(rsum[:], ssum[:])
                    o_sb = s3p.tile([128, A], f32, tag="osb")
                    nc.vector.tensor_scalar(out=o_sb[:], in0=e_sb[:],
                                            scalar1=rsum[:], scalar2=None,
                                            op0=ALU.mult)
                    nc.sync.dma_start(out_t[t * 128: t * 128 + rows, :],
                                      o_sb[:rows, :])

    nc.compile()
    return nc


# --------------------------------------------------------------------------
# public entry point
# --------------------------------------------------------------------------

def run(inputs, trace=False):
    global last_results
    x = np.asarray(inputs["x"], np.float32)
    edge_index = np.asarray(inputs["edge_index"])
    N, D = x.shape
    H = np.asarray(inputs["W"]).shape[1]
    A = np.asarray(inputs["W3"]).shape[1]
    assert N % NCORES == 0
    NLOC = N // NCORES

    per_core, NA, NB, ginfo, blk_of, TOTB = _prep_edges(edge_index, N, NLOC)

    key = (N, D, H, A, NLOC, tuple(NA), tuple(NB))
    if _cache.get("key") != key:
        _cache["nc"] = _build_nc(N, D, H, A, NLOC, NA, NB, ginfo, blk_of, TOTB)
        _cache["key"] = key
    nc = _cache["nc"]

    g = lambda k: np.ascontiguousarray(np.asarray(inputs[k], np.float32))
    common = {
        "W": g("W"),
        "asrc_b": np.tile(g("a_src")[None, :], (128, 1)),
        "adst_b": np.tile(g("a_dst")[None, :], (128, 1)),
        "b_gat": g("b_gat").reshape(H, 1),
        "bn0p": np.stack([g("g0"), g("beta0")], 1),
        "bn2p": np.stack([g("g2"), g("beta2")], 1),
        "W1": g("W1"), "b1": g("b1").reshape(H, 1),
        "W2": g("W2"), "b2": g("b2").reshape(H, 1),
        "W3": g("W3"), "b3": g("b3").reshape(A, 1),
        "ident": np.eye(128, dtype=np.float32),
        "ones_row": np.ones((1, 128), np.float32),
    }
    in_maps = []
    for i in range(NCORES):
        m = dict(common)
        xs = x[i * NLOC:(i + 1) * NLOC]
        m["xT_shard"] = np.ascontiguousarray(xs.T)
        m["src_idx"] = per_core[i]["src_idx"]
        m["P"] = per_core[i]["P"]
        in_maps.append(m)

    last_results = run_bass_kernel_spmd(nc, in_maps, list(range(NCORES)),
                                        trace=trace)
    out = np.concatenate([last_results.results[i]["out"] for i in range(NCORES)], 0)
    return np.ascontiguousarray(out)


def kernel(**inputs) -> np.ndarray:
    return run(inputs, trace=False)
